# revision 36
# baseline (speedup 1.0000x reference)
"""Causal self-attention kernel for Trainium2, 8 NeuronCores, data-parallel over batch.

v7: 2-head-packed score matmuls, GPSIMD mask, yT-layout denominators with
fused divide-normalize, single-matmul projection, bf16 transposed output.

Problem: B=4096 independent attentions, T=64, DIM=128, 4 heads of 32;
y = proj(attn(x)). k_in / v_in unused (overwritten internally) -> never shipped.

Per core: 512 batches = 32768 tokens, 64 megas of 512 tokens (8 batches).
Per mega (PE cycles, bf16 1 cyc/col):
  - qkv: qt/kt [dim, tok] via const-W stationary (512+512); v natural [tok, dim]
    via xT-chunk stationary (512).
  - scores: TWO heads per matmul: stationary ktE block-diag [64 K, 128 M]
    (M = 2 heads x 64 keys), moving qtA natural 64-row windows -> out
    [128 = 2h x keys, 64 q]; 16 matmuls x 64 cols = 1024.
  - mask: multiplicative 0/1 bf16 pattern on GPSIMD after one full-width exp.
  - rbY: denominators broadcast directly in yT row layout (ones stationary,
    attn moving; 2 x 512). yt: 32 matmuls x 64 = 2048. proj: ONE 512-col
    matmul (const Wp stationary) -> yfT [dout, tok].
  - normalize fused into yt eviction: ytS = yt_ps / rbY_ps (DVE divide).
  - output: ACT Identity+bias eviction to bf16, DMA [dout, tok]; host
    transposes and upcasts.
Evictions balanced across DVE/ACT/Pool; ktE built with 4x-mode bf16 copies.
Schedule: 3-stage software pipeline as v3; x^T prefetched 8 megas ahead.
PSUM: 8 banks static (qt, kt, v, sc x2, rbY, yt, yfT).
"""

import sys

for _p in ("/opt/trn_rl_repo", "/root/.axon_site/_ro/trn_rl_repo"):
    if _p not in sys.path:
        sys.path.insert(0, _p)

from contextlib import ExitStack

import ml_dtypes
import numpy as np

import concourse.bass as bass
import concourse.tile as tile
from concourse import bacc
from concourse import mybir
from concourse.bass_utils import run_bass_kernel_spmd

F32 = mybir.dt.float32
BF16 = mybir.dt.bfloat16

B, T, D, H, HS = 4096, 64, 128, 4, 32
NCORES = 8
BC = B // NCORES            # 512 batches per core
TOK = BC * T                # 32768 tokens per core
MEGA = 512                  # tokens per mega-tile (8 batches)
SCALE = 1.0 / float(np.sqrt(HS))

_CACHE = {}
LAST_RESULT = None
STAGE_LOG = []  # (stage, mega, first_inst_idx, last_inst_idx) — profiling aid


def _bf16(a):
    return np.asarray(a, dtype=np.float32).astype(ml_dtypes.bfloat16)


def _host_consts(W_attn, b_attn, W_proj, b_proj):
    """bf16 pack cb [128, *] and fp32 pack cf [128, *]."""
    Wq = np.ascontiguousarray(W_attn[:, 0:128] * SCALE)          # [128,128]
    Wk = np.ascontiguousarray(W_attn[:, 128:256])                # k-bias dropped
    Wv = np.ascontiguousarray(W_attn[:, 256:384])
    Wp = np.ascontiguousarray(W_proj)                            # [d, dout]
    bqs = b_attn[0:128] * SCALE
    bv = b_attn[256:384]
    bp_eff = b_proj + bv @ W_proj                                # [128] (dout)

    # onesY [128, 64]: [s'*64+k, 32*s+i] = (s' == s)
    onesY = np.zeros((128, 64), dtype=np.float32)
    onesY[0:64, 0:32] = 1.0
    onesY[64:128, 32:64] = 1.0

    # Causal mask via matmul K-extension: score[m, n] += sum_j A[j, m]*U[j, n]
    # = NEG * [key(m) > q(n)].  U[j, n] = [j > n]; A[j, m] = NEG*[m%64 == j].
    NEG = -80.0
    jj = np.arange(64).reshape(64, 1)
    nn = np.arange(64).reshape(1, 64)
    u0 = (jj > nn).astype(np.float32)                            # [64, 64]
    # uinit [128, 512]: U tiled over 8 batch blocks, at BOTH row halves
    uinit = np.tile(np.tile(u0, (1, 8)), (2, 1))                 # [128, 512]
    mm = np.arange(128).reshape(1, 128)
    a0 = NEG * (mm % 64 == jj).astype(np.float32)                # [64, 128]
    a_t = np.tile(a0, (1, 8))                                    # [64, 1024]
    # ktEinit [128, 2048]: X0 cols 0:1024 (A at rows 64:128), X1 cols
    # 1024:2048 (A at rows 0:64); zeros elsewhere (incl. data blocks).
    ktEinit = np.zeros((128, 2048), dtype=np.float32)
    ktEinit[64:128, 0:1024] = a_t
    ktEinit[0:64, 1024:2048] = a_t

    cb_parts = [("wq", Wq), ("wk", Wk), ("wv", Wv), ("wp", Wp),
                ("onesY", onesY), ("uinit", uinit), ("ktEinit", ktEinit)]
    cb = np.concatenate([np.asarray(a, dtype=np.float32) for _, a in cb_parts],
                        axis=1).astype(ml_dtypes.bfloat16)
    cb_off = {}
    off = 0
    for name, a in cb_parts:
        cb_off[name] = (off, a.shape[1])
        off += a.shape[1]

    # fp32: per-partition scalars: bq (qdim rows), bpE (dout rows)
    cf_parts = [("bq", bqs.reshape(128, 1)), ("bpE", bp_eff.reshape(128, 1))]
    cf = np.concatenate([a for _, a in cf_parts], axis=1).astype(np.float32)
    cf_off = {}
    off = 0
    for name, a in cf_parts:
        cf_off[name] = (off, a.shape[1])
        off += a.shape[1]
    return cb, cb_off, cf, cf_off


def _build_program(cb_off, cb_cols, cf_off, cf_cols, ntok=TOK):
    nmega = ntok // MEGA
    nc = bacc.Bacc()
    x_p = nc.declare_dram_parameter("x", [nmega * 128, MEGA], BF16, isOutput=False)
    # output transposed per mega: [dout, tok]; host untransposes
    y_p = nc.declare_dram_parameter("y", [nmega * 128, MEGA], BF16, isOutput=True)
    cb_p = nc.declare_dram_parameter("cb", [128, cb_cols], BF16, isOutput=False)
    cf_p = nc.declare_dram_parameter("cf", [128, cf_cols], F32, isOutput=False)

    xT_rows = x_p.rearrange("(m d) t -> m d t", d=128)
    yT_rows = y_p.rearrange("(m d) t -> m d t", d=128)

    Copy = mybir.ActivationFunctionType.Copy
    Identity = mybir.ActivationFunctionType.Identity
    Exp = mybir.ActivationFunctionType.Exp
    MULT = mybir.AluOpType.mult

    with nc.allow_low_precision(reason="bf16 attention dataflow"), \
            tile.TileContext(nc) as tc, ExitStack() as ctx:
        cpool = ctx.enter_context(tc.tile_pool(name="consts", bufs=1))
        sb = ctx.enter_context(tc.tile_pool(name="sb", bufs=2))
        ps = ctx.enter_context(tc.tile_pool(name="ps", bufs=1, space="PSUM"))

        cball = cpool.tile([128, cb_cols], BF16, tag="cb_all")
        nc.sync.dma_start(out=cball[:], in_=cb_p[:])
        cfall = cpool.tile([128, cf_cols], F32, tag="cf_all")
        nc.sync.dma_start(out=cfall[:], in_=cf_p[:])
        CB = {n: cball[:, o: o + w] for n, (o, w) in cb_off.items()}
        CF = {n: cfall[:, o: o + w] for n, (o, w) in cf_off.items()}

        PREFETCH = 8
        KTE_BUFS = 3
        xT_tiles = {}

        def fetch_x(mm):
            if mm >= nmega:
                return
            t = sb.tile([128, MEGA], BF16, tag="xT", bufs=PREFETCH + 1, name=f"xT{mm}")
            nc.sync.dma_start(out=t[:], in_=xT_rows[mm])
            xT_tiles[mm] = t

        for mm in range(PREFETCH):
            fetch_x(mm)

        # initialize rotating buffers once: ktE gets mask rows A + zeros,
        # qt0/qt1 get the U rows (data halves are overwritten every mega)
        for zz in range(KTE_BUFS):
            zt = sb.tile([128, 2, 8, 128], BF16, tag="ktE", bufs=KTE_BUFS, name=f"ktEz{zz}")
            nc.vector.tensor_copy(
                zt[:], CB["ktEinit"].rearrange("p (x b m) -> p x b m", x=2, b=8))
        for tag in ("qt0", "qt1"):
            for zz in range(3):
                ut = sb.tile([128, 512], BF16, tag=tag, bufs=3, name=f"{tag}z{zz}")
                nc.vector.tensor_copy(ut[:], CB["uinit"])

        def qkv_stage(m):
            xT = xT_tiles.pop(m)
            # q^T [qdim, tok]; bias folded into the split evictions
            qt_ps = ps.tile([128, 512], F32, tag="qt", bufs=2)
            nc.tensor.matmul(qt_ps[:], CB["wq"], xT[:], start=True, stop=True)
            # qt0: heads 0,1 data at rows 0:64, const U rows at 64:128
            # qt1: const U rows at 0:64, heads 2,3 data at rows 64:128
            qt0 = sb.tile([128, 512], BF16, tag="qt0", bufs=3)
            nc.vector.tensor_scalar_add(qt0[0:64, :], qt_ps[0:64, :], CF["bq"][0:64])
            qt1 = sb.tile([128, 512], BF16, tag="qt1", bufs=3)
            nc.scalar.activation(qt1[64:128, :], qt_ps[64:128, :], Identity, bias=CF["bq"][64:128])
            # k^T [kdim, tok] and v share one 2-bank PSUM tile -> one eviction
            kv_ps = ps.tile([128, 1024], F32, tag="kv")
            nc.tensor.matmul(kv_ps[:, 0:512], CB["wk"], xT[:], start=True, stop=True)
            # v natural [tok, d] per 128-token chunk, into the kv tile
            v_view = kv_ps[:, 512:1024].rearrange("p (c d) -> p c d", c=4)
            for c in range(4):
                nc.tensor.matmul(
                    v_view[:, c, :], xT[:, c * 128:(c + 1) * 128], CB["wv"],
                    start=True, stop=True,
                )
            kvN = sb.tile([128, 1024], BF16, tag="kvN", bufs=4)
            nc.scalar.activation(kvN[:], kv_ps[:], Copy)
            ktN = kvN[:, 0:512].rearrange("p (b k) -> p b k", b=8)
            v_s = kvN[:, 512:1024].rearrange("p (c d) -> p c d", c=4)
            # ktE block-diag [128, 2 X, 8 bb, 128 m]: X0 = data rows 0:64 +
            # mask rows A at 64:128; X1 = A at 0:64 + data rows 64:128.
            # Head h data block -> rows 32h:32h+32, col half h%2.
            ktE = sb.tile([128, 2, 8, 128], BF16, tag="ktE", bufs=KTE_BUFS)
            for h in range(4):
                r0 = 32 * h
                c0 = 64 * (h % 2)
                eng = nc.vector if h == 0 else nc.gpsimd
                eng.tensor_copy(ktE[r0:r0 + 32, h // 2, :, c0:c0 + 64], ktN[r0:r0 + 32])
            # v_sw = v_s with partition halves swapped (for yt matmuls where
            # the attn row-half parity differs from the batch parity)
            v_sw = sb.tile([128, 4, 128], BF16, tag="v_sw", bufs=4)
            nc.sync.dma_start(out=v_sw[0:64, :, :], in_=v_s[64:128])
            nc.sync.dma_start(out=v_sw[64:128, :, :], in_=v_s[0:64])
            return {"qt0": qt0, "qt1": qt1, "ktE": ktE, "v_s": v_s, "v_sw": v_sw}

        def score_stage(qk):
            """Packed score matmuls (mask folded via K-extension) + exp (B)."""
            qts = (qk["qt0"], qk["qt1"])
            ktE = qk["ktE"]
            sc_ps = ps.tile([128, 2, 8, 64], F32, tag="sc")
            for X in range(2):
                for bb in range(8):
                    nc.tensor.matmul(
                        sc_ps[:, X, bb, :],
                        ktE[:, X, bb, :],
                        qts[X][:, bb * 64:(bb + 1) * 64],
                        start=True, stop=True,
                    )
            attn_u = sb.tile([128, 1024], BF16, tag="attn_u", bufs=4)
            nc.scalar.activation(attn_u[:], sc_ps[:].rearrange("p x b q -> p (x b q)"), Exp)
            return attn_u

        def attnv_stage(qk, attn_u):
            """yT-layout denominators + attn@v + fused normalize (stage D)."""
            rb_ps = ps.tile([128, 512], F32, tag="rbY")
            for X in range(2):
                nc.tensor.matmul(
                    rb_ps[64 * X:64 * X + 64, :], CB["onesY"],
                    attn_u[:, 512 * X:512 * X + 512],
                    start=True, stop=True,
                )
            yt_ps = ps.tile([128, 512], F32, tag="yt", name="yt")
            for h in range(4):
                X, s = h // 2, h % 2
                for bb in range(8):
                    # operand bases must match: batch bb's keys sit at rows
                    # 64*(bb%2) of v_s; attn rows for head h sit at 64*s.
                    # s == bb%2 -> v_s, else the half-swapped copy v_sw.
                    vt = qk["v_s"] if s == (bb % 2) else qk["v_sw"]
                    nc.tensor.matmul(
                        yt_ps[32 * h:32 * h + 32, bb * 64:(bb + 1) * 64],
                        vt[64 * s:64 * s + 64, bb // 2, 32 * h:32 * h + 32],
                        attn_u[64 * s:64 * s + 64, 512 * X + 64 * bb: 512 * X + 64 * bb + 64],
                        start=True, stop=True,
                        tile_position=(64 * s, 32 * h),
                    )
            rec = sb.tile([128, 512], BF16, tag="rec", bufs=3)
            nc.vector.reciprocal(rec[:], rb_ps[:])
            ytS = sb.tile([128, 512], BF16, tag="ytS", bufs=3)
            nc.vector.tensor_tensor(ytS[:], yt_ps[:], rec[:], op=MULT)
            return ytS

        def proj_stage(ytS):
            """Projection: yfT [dout, tok] = Wp^T @ ytS (stage E)."""
            yf_ps = ps.tile([128, 512], F32, tag="rbY", name="yfT")
            nc.tensor.matmul(yf_ps[:], CB["wp"], ytS[:], start=True, stop=True)
            return yf_ps

        def yout_stage(m, yf_ps):
            """Bias + bf16 eviction + DMA out (stage F)."""
            y_out = sb.tile([128, 512], BF16, tag="y_out", bufs=3)
            nc.vector.tensor_scalar_add(y_out[:], yf_ps[:], CF["bpE"])
            nc.scalar.dma_start(out=yT_rows[m], in_=y_out[:])

        # 6-stage software pipeline; emission order per iteration is
        # F, E, A, B, D so every cross-engine consumer reads data >= 1
        # iteration old (exp output gets 2 iterations of slack before
        # rbY/yt consume it) and PSUM WARs resolve against already-
        # emitted readers.
        st_qk = {}     # A outputs, consumed by B (sc) and D (yt)
        st_au = {}     # B output attn_u, consumed by D
        st_ys = {}     # D output ytS, consumed by E
        st_yf = {}     # E output yf_ps, consumed by F

        def _log(stage, m, i0):
            STAGE_LOG.append((stage, m, i0, nc.next_id()))

        for i in range(nmega + 5):
            P = 0.0030
            if i >= 5:
                tc.tile_set_cur_wait(i * P)
                i0 = nc.next_id()
                yout_stage(i - 5, st_yf.pop(i - 5))
                _log("F", i - 5, i0)
            if i < nmega:
                tc.tile_set_cur_wait(i * P + 0.1 * P)
                i0 = nc.next_id()
                fetch_x(i + PREFETCH)
                st_qk[i] = qkv_stage(i)
                _log("A", i, i0)
            if 1 <= i and i - 1 < nmega:
                tc.tile_set_cur_wait(i * P + 0.35 * P)
                i0 = nc.next_id()
                st_au[i - 1] = score_stage(st_qk[i - 1])
                _log("B", i - 1, i0)
            if 3 <= i and i - 3 < nmega:
                tc.tile_set_cur_wait(i * P + 0.6 * P)
                i0 = nc.next_id()
                st_ys[i - 3] = attnv_stage(st_qk.pop(i - 3), st_au.pop(i - 3))
                _log("D", i - 3, i0)
            if 4 <= i and i - 4 < nmega:
                tc.tile_set_cur_wait(i * P + 0.85 * P)
                i0 = nc.next_id()
                st_yf[i - 4] = proj_stage(st_ys.pop(i - 4))
                _log("E", i - 4, i0)
    nc.compile()
    return nc


def _cast_bf16_fast(x):
    """fp32 -> bf16 round-to-nearest-even via bit ops (faster than astype)."""
    u = x.view(np.uint32)
    r = ((u >> 16) & 1) + np.uint32(0x7FFF)
    return ((u + r) >> 16).astype(np.uint16).view(ml_dtypes.bfloat16)


def kernel(x, k_in, v_in, W_attn, b_attn, W_proj, b_proj):
    x = np.asarray(x, dtype=np.float32)
    cb, cb_off, cf, cf_off = _host_consts(
        np.asarray(W_attn, dtype=np.float32),
        np.asarray(b_attn, dtype=np.float32),
        np.asarray(W_proj, dtype=np.float32),
        np.asarray(b_proj, dtype=np.float32),
    )
    key = "prog"
    if key not in _CACHE:
        _CACHE[key] = _build_program(cb_off, cb.shape[1], cf_off, cf.shape[1])
    nc = _CACHE[key]

    xb = _cast_bf16_fast(np.ascontiguousarray(x.reshape(B * T, D)))
    # pre-transpose per mega on host: [TOK, 128] -> [nmega, 128, 512]
    xbt = np.ascontiguousarray(
        xb.reshape(NCORES, TOK // MEGA, MEGA, D).transpose(0, 1, 3, 2)
    )
    in_maps = []
    for i in range(NCORES):
        shard = xbt[i].reshape((TOK // MEGA) * 128, MEGA)
        in_maps.append({"x": shard, "cb": cb, "cf": cf})

    res = run_bass_kernel_spmd(nc, in_maps, list(range(NCORES)))
    global LAST_RESULT
    LAST_RESULT = res
    outs = []
    for i in range(NCORES):
        yT = np.asarray(res.results[i]["y"]).view(ml_dtypes.bfloat16)
        yT = yT.reshape(TOK // MEGA, 128, MEGA).transpose(0, 2, 1)  # [m, tok, d]
        outs.append(yT.astype(np.float32).reshape(BC, T, D))
    return np.concatenate(outs, axis=0)


if __name__ == "__main__":
    rng = np.random.default_rng(0)
    Bs = 64  # small smoke test: one core, 8 megas
    ntok = Bs * T
    xs = rng.standard_normal((Bs, T, D), dtype=np.float32)
    Wa = rng.standard_normal((D, 3 * D), dtype=np.float32) / np.sqrt(D)
    ba = rng.standard_normal(3 * D, dtype=np.float32) * 0.01
    Wp = rng.standard_normal((D, D), dtype=np.float32) / np.sqrt(D)
    bp = rng.standard_normal(D, dtype=np.float32) * 0.01

    cb, cb_off, cf, cf_off = _host_consts(Wa, ba, Wp, bp)
    nc = _build_program(cb_off, cb.shape[1], cf_off, cf.shape[1], ntok=ntok)
    xb = _cast_bf16_fast(np.ascontiguousarray(xs.reshape(ntok, D)))
    xbt = np.ascontiguousarray(
        xb.reshape(ntok // MEGA, MEGA, D).transpose(0, 2, 1)
    ).reshape((ntok // MEGA) * 128, MEGA)
    res = run_bass_kernel_spmd(nc, [{"x": xbt, "cb": cb, "cf": cf}], [0])
    yT = np.asarray(res.results[0]["y"]).view(ml_dtypes.bfloat16)
    y = yT.reshape(ntok // MEGA, 128, MEGA).transpose(0, 2, 1).astype(np.float32)
    y = y.reshape(Bs, T, D)

    # numpy reference
    def ref(x):
        qkv = x @ Wa + ba
        q, k, v = np.split(qkv, 3, axis=2)

        def heads(u):
            return u.reshape(Bs, T, H, HS).transpose(0, 2, 1, 3)

        q, k, v = heads(q), heads(k), heads(v)
        s = np.einsum('bhqd,bhkd->bhqk', q, k) / np.sqrt(HS)
        mask = np.tril(np.ones((T, T), dtype=bool))
        s = np.where(mask, s, -1e30)
        e = np.exp(s - s.max(axis=-1, keepdims=True))
        a = e / e.sum(axis=-1, keepdims=True)
        o = np.einsum('bhqk,bhkd->bhqd', a, v)
        o = o.transpose(0, 2, 1, 3).reshape(Bs, T, D)
        return o @ Wp + bp

    want = ref(xs)
    err = np.linalg.norm(y - want) / np.linalg.norm(want)
    print("smoke rel err:", err)
